# revision 16
# baseline (speedup 1.0000x reference)
"""Multi-head causal self-attention (B=2, S=2048, D=1024, H=16) on 8 trn2 cores.

Sharding: 2-way data-parallel over batch x 4-way tensor-parallel over heads.
Core c handles batch b=c//4 and heads [4*(c%4), 4*(c%4)+4).

Per-core device program (all fp32 data, fp32r matmuls):
  1. QKV projections from host-pre-transposed x^T and W^T shards.
     Q^T,K^T produced as [head-channel, token]; V as [token, channel] with a
     fused ones-column (denominator trick).
  2. Flash-style causal attention per (t-block 512, head): scores^T tile
     [u=128, t<=512] -> exp (ScalarE, scale=1/8) -> diag mask -> AV-accumulate
     into PSUM [65, 512] whose row 64 is the softmax denominator.
     Softmax normalization is applied after AV (per-head, exact).
  3. Local o_proj partial product over this core's 256 v-dims.
Host sums the 4 per-batch partials (the v-contraction all-reduce) and stacks.
"""

import numpy as np
from contextlib import ExitStack

import concourse.bass as bass
import concourse.bacc as bacc
import concourse.tile as tile
import concourse.mybir as mybir
from concourse.bass_utils import run_bass_kernel_spmd

F32 = mybir.dt.float32
F32R = mybir.dt.float32r
BF16 = mybir.dt.bfloat16
EXP = mybir.ActivationFunctionType.Exp

B, S, D = 2, 2048, 1024
H, HS = 16, 64
NCORES = 8
HPC = H // (NCORES // B)  # heads per core = 4
KD = HPC * HS             # per-core projected dims = 256
NKK = KD // 128           # head-dim partition chunks = 2
NDC = D // 128            # contraction chunks = 8
TB = 512                  # t-block width
NTB = S // TB             # 4
NUT = S // 128            # u-tiles = 16
SCALE = float(HS) ** -0.5


def _r(ap):
    """fp32r view of an fp32 AP (full-rate PE matmul for N>=256)."""
    return ap.bitcast(F32R)


def build_program(mm_dt=F32R):
    rr = lambda ap: ap
    nc = bacc.Bacc("TRN2", target_bir_lowering=False, debug=False)
    xt = nc.dram_tensor("xt", [D, S], F32R, kind="ExternalInput").ap()
    wqt = nc.dram_tensor("wqt", [D, KD], F32R, kind="ExternalInput").ap()
    wkt = nc.dram_tensor("wkt", [D, KD], F32R, kind="ExternalInput").ap()
    wvt = nc.dram_tensor("wvt", [D, KD], F32R, kind="ExternalInput").ap()
    wot = nc.dram_tensor("wot", [KD, D], F32R, kind="ExternalInput").ap()
    maskd = nc.dram_tensor("mask", [128, 128], F32R, kind="ExternalInput").ap()
    y = nc.dram_tensor("y", [S, D], F32, kind="ExternalOutput").ap()

    with tile.TileContext(nc) as tc, ExitStack() as ctx:
        wpool = ctx.enter_context(tc.tile_pool(name="w", bufs=1))
        big = ctx.enter_context(tc.tile_pool(name="big", bufs=1))
        xtg_pool = ctx.enter_context(tc.tile_pool(name="xtg", bufs=32))
        e_pool = ctx.enter_context(tc.tile_pool(name="expS", bufs=4))
        sm_pool = ctx.enter_context(tc.tile_pool(name="small", bufs=3))
        ypool = ctx.enter_context(tc.tile_pool(name="yout", bufs=3))
        psS = ctx.enter_context(tc.tile_pool(name="psS", bufs=2, space="PSUM"))
        psM = ctx.enter_context(tc.tile_pool(name="psM", bufs=2, space="PSUM"))
        psO = ctx.enter_context(tc.tile_pool(name="psO", bufs=2, space="PSUM"))

        # --- persistent SBUF tensors ---
        wq_sb = wpool.tile([128, NDC, KD], F32R)
        wk_sb = wpool.tile([128, NDC, KD], F32R)
        wv_sb = wpool.tile([128, NDC, KD], F32R)
        wo_sb = wpool.tile([128, NKK, D], F32R)
        mask_sb = wpool.tile([128, 128], F32R)
        QT = big.tile([128, NKK, S], F32R)   # [channel(2 heads), kk, token]
        KT = big.tile([128, NKK, S], F32R)
        VA = big.tile([128, HPC, NUT, HS + 1], F32R)  # [tok, head, utile, ch|1]
        OT = big.tile([128, NKK, S], F32R)   # normalized attention out^T

        nc.sync.dma_start(wq_sb[:], wqt.rearrange("(c p) k -> p c k", p=128))
        nc.sync.dma_start(wk_sb[:], wkt.rearrange("(c p) k -> p c k", p=128))
        nc.sync.dma_start(wv_sb[:], wvt.rearrange("(c p) k -> p c k", p=128))
        nc.sync.dma_start(wo_sb[:], wot.rearrange("(c p) d -> p c d", p=128))
        nc.sync.dma_start(mask_sb[:], maskd)
        # mask row 0 is all-ones (u=0 <= every t); col 127 likewise.
        nc.vector.tensor_copy(
            VA[:, :, :, HS], mask_sb[:, 127:128].to_broadcast([128, HPC, NUT])
        )

        xts = [[None] * NTB for _ in range(NDC)]
        for c in range(NDC):
            for tg in range(NTB):
                t = xtg_pool.tile([128, TB], F32R, tag="xtg")
                nc.sync.dma_start(
                    t[:], xt[128 * c:128 * (c + 1), TB * tg:TB * (tg + 1)]
                )
                xts[c][tg] = t

        def qkv(tg):
            """Projections for t-group tg: Q^T/K^T columns, V u-tiles."""
            for w_sb, dst in ((wq_sb, QT), (wk_sb, KT)):
                for kk in range(NKK):
                    ps = psM.tile([128, TB], F32, tag="m512", name="qk_ps")
                    for c in range(NDC):
                        nc.tensor.matmul(
                            ps[:],
                            rr(w_sb[:, c, 128 * kk:128 * (kk + 1)]),
                            rr(xts[c][tg][:]),
                            start=(c == 0), stop=(c == NDC - 1),
                        )
                    nc.scalar.copy(dst[:, kk, TB * tg:TB * (tg + 1)], ps[:])
            for tt in range(TB // 128):
                ps = psM.tile([128, KD], F32, tag="m512", name="v_ps")
                for c in range(NDC):
                    nc.tensor.matmul(
                        ps[:],
                        rr(xts[c][tg][:, 128 * tt:128 * (tt + 1)]),
                        rr(wv_sb[:, c, :]),
                        start=(c == 0), stop=(c == NDC - 1),
                    )
                ut = (TB // 128) * tg + tt
                for h in range(HPC):
                    nc.vector.tensor_copy(
                        VA[:, h, ut, 0:HS], ps[:, HS * h:HS * (h + 1)]
                    )

        def attention(tb):
            """Causal attention for t-block tb, heads paired per chunk, AV
            trailing one u-step behind scores; then o_proj for the block."""
            nut = 4 * tb + 4
            for hp in range(NKK):
                Os = [psO.tile([HS + 1, TB], F32, tag="av", name=f"O{g}")
                      for g in range(2)]

                def av_pair(pes, ptoff, pk, stop):
                    for g in range(2):
                        nc.tensor.matmul(
                            Os[g][:, ptoff:],
                            rr(VA[:, 2 * hp + g, pk, :]),
                            rr(pes[:, g, ptoff:]),
                            start=(pk == 0), stop=stop,
                        )

                pend = None
                for k in range(nut):
                    toff = max(0, 128 * (k - 4 * tb))
                    sp = psS.tile([128, 2, TB], F32, tag="sp")
                    for g in range(2):
                        nc.tensor.matmul(
                            sp[:, g, toff:],
                            rr(KT[64 * g:64 * g + 64, hp, 128 * k:128 * (k + 1)]),
                            rr(QT[64 * g:64 * g + 64, hp,
                                  TB * tb + toff:TB * (tb + 1)]),
                            start=True, stop=True,
                        )
                    es = e_pool.tile([128, 2, TB], F32R, tag="expS")
                    nc.scalar.activation(es[:, :, toff:], sp[:, :, toff:], EXP,
                                         scale=SCALE)
                    if k >= 4 * tb:  # diagonal: zero the u>t triangle
                        for g in range(2):
                            nc.vector.tensor_mul(
                                es[:, g, toff:toff + 128],
                                es[:, g, toff:toff + 128], mask_sb[:]
                            )
                    if pend is not None:
                        av_pair(*pend, stop=False)
                    pend = (es, toff, k)
                av_pair(*pend, stop=True)
                for g in range(2):
                    ro = 64 * g
                    den = sm_pool.tile([1, TB], F32R, tag="den")
                    nc.vector.tensor_copy(den[:], Os[g][HS:HS + 1, :])
                    bc_ps = psM.tile([64, TB], F32, tag="m512", name="bc_ps")
                    nc.tensor.matmul(bc_ps[:], mask_sb[0:1, 0:64], den[:],
                                     start=True, stop=True)
                    bc = sm_pool.tile([64, TB], F32, tag="bc_sb")
                    nc.vector.reciprocal_approx_fast(bc[:], bc_ps[:])
                    nc.vector.tensor_mul(
                        OT[ro:ro + 64, hp, TB * tb:TB * (tb + 1)],
                        Os[g][0:HS, :], bc[:]
                    )
            for i in range(4 * tb, 4 * tb + 4):
                for j in range(D // 512):
                    ps = psM.tile([128, 512], F32, tag="m512", name="yps")
                    for vc in range(NKK):
                        nc.tensor.matmul(
                            ps[:],
                            rr(OT[:, vc, 128 * i:128 * (i + 1)]),
                            rr(wo_sb[:, vc, 512 * j:512 * (j + 1)]),
                            start=(vc == 0), stop=(vc == NKK - 1),
                        )
                    yt = ypool.tile([128, 512], F32, tag="yt")
                    nc.vector.tensor_copy(yt[:], ps[:])
                    nc.sync.dma_start(
                        y[128 * i:128 * (i + 1), 512 * j:512 * (j + 1)], yt[:]
                    )

        # Interleave: attention(tb) only needs t-groups <= tb, so QKV(tg)
        # followed by attention(tg) keeps ScalarE exp work overlapped with
        # the next t-group's projection matmuls.
        for tg in range(NTB):
            qkv(tg)
            attention(tg)

    nc.compile()
    return nc


def make_in_maps(x, q_w, k_w, v_w, o_w):
    x = np.asarray(x, dtype=np.float32)
    mask = np.triu(np.ones((128, 128), dtype=np.float32))  # keep where u <= t
    xtb = [np.ascontiguousarray(x[b].T) for b in range(B)]
    in_maps = []
    for c in range(NCORES):
        b, hg = divmod(c, NCORES // B)
        sl = slice(hg * KD, (hg + 1) * KD)
        in_maps.append({
            "xt": xtb[b],
            "wqt": np.ascontiguousarray(np.asarray(q_w, np.float32)[sl, :].T),
            "wkt": np.ascontiguousarray(np.asarray(k_w, np.float32)[sl, :].T),
            "wvt": np.ascontiguousarray(np.asarray(v_w, np.float32)[sl, :].T),
            "wot": np.ascontiguousarray(np.asarray(o_w, np.float32)[:, sl].T),
            "mask": mask,
        })
    return in_maps


def combine_outputs(results):
    """results: list of 8 dicts with per-core partial y [S, D]."""
    per_b = NCORES // B
    ys = [np.asarray(results[c]["y"], dtype=np.float32) for c in range(NCORES)]
    out = np.stack(
        [sum(ys[b * per_b + i] for i in range(per_b)) for b in range(B)]
    )
    return np.ascontiguousarray(out, dtype=np.float32)


_PROGRAM = None


def kernel(x, q_proj_weight, k_proj_weight, v_proj_weight, o_proj_weight,
           **extra):
    global _PROGRAM
    if _PROGRAM is None:
        _PROGRAM = build_program()
    in_maps = make_in_maps(x, q_proj_weight, k_proj_weight, v_proj_weight,
                           o_proj_weight)
    res = run_bass_kernel_spmd(_PROGRAM, in_maps, list(range(NCORES)))
    return combine_outputs(res.results)


if __name__ == "__main__":
    nc = build_program()
    n = len(nc.m.functions[0].blocks[0].instructions) if nc.m.functions else -1
    print("program built")


# revision 17
# speedup vs baseline: 1.0923x; 1.0923x over previous
"""Multi-head causal self-attention (B=2, S=2048, D=1024, H=16) on 8 trn2 cores.

Sharding: 2-way data-parallel over batch x 4-way tensor-parallel over heads.
Core c handles batch b=c//4 and heads [4*(c%4), 4*(c%4)+4).

Per-core device program (all fp32 data, fp32r matmuls):
  1. QKV projections from host-pre-transposed x^T and W^T shards.
     Q^T,K^T produced as [head-channel, token]; V as [token, channel] with a
     fused ones-column (denominator trick).
  2. Flash-style causal attention per (t-block 512, head): scores^T tile
     [u=128, t<=512] -> exp (ScalarE, scale=1/8) -> diag mask -> AV-accumulate
     into PSUM [65, 512] whose row 64 is the softmax denominator.
     Softmax normalization is applied after AV (per-head, exact).
  3. Local o_proj partial product over this core's 256 v-dims.
Host sums the 4 per-batch partials (the v-contraction all-reduce) and stacks.
"""

import numpy as np
from contextlib import ExitStack

import concourse.bass as bass
import concourse.bacc as bacc
import concourse.tile as tile
import concourse.mybir as mybir
from concourse.bass_utils import run_bass_kernel_spmd

F32 = mybir.dt.float32
F32R = mybir.dt.float32r
BF16 = mybir.dt.bfloat16
EXP = mybir.ActivationFunctionType.Exp

B, S, D = 2, 2048, 1024
H, HS = 16, 64
NCORES = 8
HPC = H // (NCORES // B)  # heads per core = 4
KD = HPC * HS             # per-core projected dims = 256
NKK = KD // 128           # head-dim partition chunks = 2
NDC = D // 128            # contraction chunks = 8
TB = 512                  # t-block width
NTB = S // TB             # 4
NUT = S // 128            # u-tiles = 16
SCALE = float(HS) ** -0.5


def _r(ap):
    """fp32r view of an fp32 AP (full-rate PE matmul for N>=256)."""
    return ap.bitcast(F32R)


def build_program(mm_dt=F32R):
    rr = lambda ap: ap
    nc = bacc.Bacc("TRN2", target_bir_lowering=False, debug=False)
    xt = nc.dram_tensor("xt", [D, S], F32R, kind="ExternalInput").ap()
    wqt = nc.dram_tensor("wqt", [D, KD], F32R, kind="ExternalInput").ap()
    wkt = nc.dram_tensor("wkt", [D, KD], F32R, kind="ExternalInput").ap()
    wvt = nc.dram_tensor("wvt", [D, KD], F32R, kind="ExternalInput").ap()
    wot = nc.dram_tensor("wot", [KD, D], F32R, kind="ExternalInput").ap()
    maskd = nc.dram_tensor("mask", [128, 128], F32R, kind="ExternalInput").ap()
    y = nc.dram_tensor("y", [S, D], F32, kind="ExternalOutput").ap()

    with tile.TileContext(nc) as tc, ExitStack() as ctx:
        wpool = ctx.enter_context(tc.tile_pool(name="w", bufs=1))
        big = ctx.enter_context(tc.tile_pool(name="big", bufs=1))
        xtg_pool = ctx.enter_context(tc.tile_pool(name="xtg", bufs=32))
        e_pool = ctx.enter_context(tc.tile_pool(name="expS", bufs=4))
        sm_pool = ctx.enter_context(tc.tile_pool(name="small", bufs=3))
        ypool = ctx.enter_context(tc.tile_pool(name="yout", bufs=3))

        # --- persistent SBUF tensors ---
        wq_sb = wpool.tile([128, NDC, KD], F32R)
        wk_sb = wpool.tile([128, NDC, KD], F32R)
        wv_sb = wpool.tile([128, NDC, KD], F32R)
        wo_sb = wpool.tile([128, NKK, D], F32R)
        mask_sb = wpool.tile([128, 128], F32R)
        QT = big.tile([128, NKK, S], F32R)   # [channel(2 heads), kk, token]
        KT = big.tile([128, NKK, S], F32R)
        VA = big.tile([128, HPC, NUT, HS + 1], F32R)  # [tok, head, utile, ch|1]
        OT = big.tile([128, NKK, S], F32R)   # normalized attention out^T

        nc.sync.dma_start(wq_sb[:], wqt.rearrange("(c p) k -> p c k", p=128))
        nc.sync.dma_start(wk_sb[:], wkt.rearrange("(c p) k -> p c k", p=128))
        nc.sync.dma_start(wv_sb[:], wvt.rearrange("(c p) k -> p c k", p=128))
        nc.sync.dma_start(wo_sb[:], wot.rearrange("(c p) d -> p c d", p=128))
        nc.sync.dma_start(mask_sb[:], maskd)
        # mask row 0 is all-ones (u=0 <= every t); col 127 likewise.
        nc.vector.tensor_copy(
            VA[:, :, :, HS], mask_sb[:, 127:128].to_broadcast([128, HPC, NUT])
        )

        xts = [[None] * NTB for _ in range(NDC)]
        for c in range(NDC):
            for tg in range(NTB):
                t = xtg_pool.tile([128, TB], F32R, tag="xtg")
                nc.sync.dma_start(
                    t[:], xt[128 * c:128 * (c + 1), TB * tg:TB * (tg + 1)]
                )
                xts[c][tg] = t

        def qkv(tg, pool):
            """Projections for t-group tg: Q^T/K^T columns, V u-tiles."""
            for w_sb, dst in ((wq_sb, QT), (wk_sb, KT)):
                for kk in range(NKK):
                    ps = pool.tile([128, TB], F32, tag="m512", name="qk_ps")
                    for c in range(NDC):
                        nc.tensor.matmul(
                            ps[:],
                            rr(w_sb[:, c, 128 * kk:128 * (kk + 1)]),
                            rr(xts[c][tg][:]),
                            start=(c == 0), stop=(c == NDC - 1),
                        )
                    nc.scalar.copy(dst[:, kk, TB * tg:TB * (tg + 1)], ps[:])
            for tt in range(TB // 128):
                ps = pool.tile([128, KD], F32, tag="m512", name="v_ps")
                for c in range(NDC):
                    nc.tensor.matmul(
                        ps[:],
                        rr(xts[c][tg][:, 128 * tt:128 * (tt + 1)]),
                        rr(wv_sb[:, c, :]),
                        start=(c == 0), stop=(c == NDC - 1),
                    )
                ut = (TB // 128) * tg + tt
                for h in range(HPC):
                    nc.vector.tensor_copy(
                        VA[:, h, ut, 0:HS], ps[:, HS * h:HS * (h + 1)]
                    )

        def attention(tb):
            """Causal attention for t-block tb, heads paired per chunk, AV
            trailing one u-step behind scores; then o_proj for the block."""
            nut = 4 * tb + 4
            for hp in range(NKK):
                Os = [psO.tile([HS + 1, TB], F32, tag="av", name=f"O{g}")
                      for g in range(2)]

                def av_pair(pes, ptoff, pk, stop):
                    for g in range(2):
                        nc.tensor.matmul(
                            Os[g][:, ptoff:],
                            rr(VA[:, 2 * hp + g, pk, :]),
                            rr(pes[:, g, ptoff:]),
                            start=(pk == 0), stop=stop,
                        )

                pend = None
                for k in range(nut):
                    toff = max(0, 128 * (k - 4 * tb))
                    sp = psS.tile([128, 2, TB], F32, tag="sp")
                    for g in range(2):
                        nc.tensor.matmul(
                            sp[:, g, toff:],
                            rr(KT[64 * g:64 * g + 64, hp, 128 * k:128 * (k + 1)]),
                            rr(QT[64 * g:64 * g + 64, hp,
                                  TB * tb + toff:TB * (tb + 1)]),
                            start=True, stop=True,
                        )
                    es = e_pool.tile([128, 2, TB], F32R, tag="expS")
                    nc.scalar.activation(es[:, :, toff:], sp[:, :, toff:], EXP,
                                         scale=SCALE)
                    if k >= 4 * tb:  # diagonal: zero the u>t triangle
                        for g in range(2):
                            nc.vector.tensor_mul(
                                es[:, g, toff:toff + 128],
                                es[:, g, toff:toff + 128], mask_sb[:]
                            )
                    if pend is not None:
                        av_pair(*pend, stop=False)
                    pend = (es, toff, k)
                av_pair(*pend, stop=True)
                for g in range(2):
                    ro = 64 * g
                    den = sm_pool.tile([1, TB], F32R, tag="den")
                    nc.vector.tensor_copy(den[:], Os[g][HS:HS + 1, :])
                    bc_ps = psM.tile([64, TB], F32, tag="m512", name="bc_ps")
                    nc.tensor.matmul(bc_ps[:], mask_sb[0:1, 0:64], den[:],
                                     start=True, stop=True)
                    bc = sm_pool.tile([64, TB], F32, tag="bc_sb")
                    nc.vector.reciprocal_approx_fast(bc[:], bc_ps[:])
                    nc.vector.tensor_mul(
                        OT[ro:ro + 64, hp, TB * tb:TB * (tb + 1)],
                        Os[g][0:HS, :], bc[:]
                    )
            for i in range(4 * tb, 4 * tb + 4):
                for j in range(D // 512):
                    ps = psM.tile([128, 512], F32, tag="m512", name="yps")
                    for vc in range(NKK):
                        nc.tensor.matmul(
                            ps[:],
                            rr(OT[:, vc, 128 * i:128 * (i + 1)]),
                            rr(wo_sb[:, vc, 512 * j:512 * (j + 1)]),
                            start=(vc == 0), stop=(vc == NKK - 1),
                        )
                    yt = ypool.tile([128, 512], F32, tag="yt")
                    nc.vector.tensor_copy(yt[:], ps[:])
                    nc.sync.dma_start(
                        y[128 * i:128 * (i + 1), 512 * j:512 * (j + 1)], yt[:]
                    )

        with tc.tile_pool(name="ps1", bufs=6, space="PSUM") as ps1:
            for tg in range(NTB):
                qkv(tg, ps1)
        psS_ctx = tc.tile_pool(name="psS", bufs=2, space="PSUM")
        psS = psS_ctx.__enter__()
        psM_ctx = tc.tile_pool(name="psM", bufs=2, space="PSUM")
        psM = psM_ctx.__enter__()
        psO_ctx = tc.tile_pool(name="psO", bufs=2, space="PSUM")
        psO = psO_ctx.__enter__()
        for tb in range(NTB):
            attention(tb)
        psO_ctx.__exit__(None, None, None)
        psM_ctx.__exit__(None, None, None)
        psS_ctx.__exit__(None, None, None)

    nc.compile()
    return nc


def make_in_maps(x, q_w, k_w, v_w, o_w):
    x = np.asarray(x, dtype=np.float32)
    mask = np.triu(np.ones((128, 128), dtype=np.float32))  # keep where u <= t
    xtb = [np.ascontiguousarray(x[b].T) for b in range(B)]
    in_maps = []
    for c in range(NCORES):
        b, hg = divmod(c, NCORES // B)
        sl = slice(hg * KD, (hg + 1) * KD)
        in_maps.append({
            "xt": xtb[b],
            "wqt": np.ascontiguousarray(np.asarray(q_w, np.float32)[sl, :].T),
            "wkt": np.ascontiguousarray(np.asarray(k_w, np.float32)[sl, :].T),
            "wvt": np.ascontiguousarray(np.asarray(v_w, np.float32)[sl, :].T),
            "wot": np.ascontiguousarray(np.asarray(o_w, np.float32)[:, sl].T),
            "mask": mask,
        })
    return in_maps


def combine_outputs(results):
    """results: list of 8 dicts with per-core partial y [S, D]."""
    per_b = NCORES // B
    ys = [np.asarray(results[c]["y"], dtype=np.float32) for c in range(NCORES)]
    out = np.stack(
        [sum(ys[b * per_b + i] for i in range(per_b)) for b in range(B)]
    )
    return np.ascontiguousarray(out, dtype=np.float32)


_PROGRAM = None


def kernel(x, q_proj_weight, k_proj_weight, v_proj_weight, o_proj_weight,
           **extra):
    global _PROGRAM
    if _PROGRAM is None:
        _PROGRAM = build_program()
    in_maps = make_in_maps(x, q_proj_weight, k_proj_weight, v_proj_weight,
                           o_proj_weight)
    res = run_bass_kernel_spmd(_PROGRAM, in_maps, list(range(NCORES)))
    return combine_outputs(res.results)


if __name__ == "__main__":
    nc = build_program()
    n = len(nc.m.functions[0].blocks[0].instructions) if nc.m.functions else -1
    print("program built")
